# revision 6
# baseline (speedup 1.0000x reference)
"""Trainium2 Bass kernel for AudioOnlyGNN (3-layer GCN + BatchNorm + mean-pool + MLP).

Sharding: nodes padded to NPAD = 8*NT*128, split contiguously across the 8
NeuronCores; each core owns edges whose dst lands in its shard.  Per layer:
  - h rows gathered from HBM with dma_gather (bf16, 128-edge blocks, indices
    precomputed/permuted on host to a tile-major row order),
  - one-hot "S panels" (dst one-hot * dinv[src]) built on VectorE with one
    tensor_scalar(is_equal, mult) vs a constant iota tile,
  - TensorE matmul accumulates [F, dst-tile] PSUM tiles (contraction = edges),
  - [dst,H] = aggT @ W on TensorE + rank-1 bias matmuls,
  - ReLU with per-partition dinv[dst] scale on ScalarE.
BatchNorm is folded into W1 plus a rank-1 shift term from on-device batch
statistics (sum / sum-of-squares via matmuls).  Mean-pool uses the same
one-hot trick per dst tile.  The 3 GCN layers run as 3 SPMD launches; between
launches the host only concatenates/permutes the 8 output shards (pure data
movement) to form the replicated full-h input of the next launch.  A tiny 4th
launch sums the per-core pooled partials and runs the classifier MLP.
"""

import sys

sys.path.insert(0, "/opt/trn_rl_repo")

import contextlib

import numpy as np
import ml_dtypes

import concourse.bacc as bacc
import concourse.bass as bass
import concourse.mybir as mybir
from concourse.tile import TileContext
from concourse.bass_utils import run_bass_kernel_spmd

BF16 = mybir.dt.bfloat16
F32 = mybir.dt.float32
I16 = mybir.dt.int16

N_CORES = 8
BN_EPS = 1e-5
TILES_PER_CHUNK = 5


# ------------------------------------------------------------------ host prep
def _wrap_idx(idx):
    n = idx.shape[0]
    assert n % 16 == 0
    a = idx.astype(np.int16).reshape(n // 16, 16).T  # [16, n/16]
    return np.tile(a, (8, 1)).copy()


def _common_schedule(src, dst, NT, SHARD, JT):
    """Per-core edge lists; per-(tile, half) block counts maxed over cores so
    the SPMD program is identical on every core."""
    per_core = []
    for c in range(N_CORES):
        base = c * SHARD
        sel = (dst >= base) & (dst < base + SHARD)
        s, d = src[sel], dst[sel]
        tile = (d - base) // 128
        dloc = (d - base) % 128
        half = ((s % 128) >= 64).astype(np.int64)
        vperm = (s % 128).astype(np.int64) * JT + (s // 128)
        vrel = np.where(half == 1, vperm - 64 * JT, vperm)
        per_core.append((tile, half, vrel, dloc, s))

    nb = np.zeros((NT, 2), np.int64)
    for tile, half, _, _, _ in per_core:
        for h in (0, 1):
            cnt = np.bincount(tile[half == h], minlength=NT)
            nb[:, h] = np.maximum(nb[:, h], (cnt + 127) // 128)
    for t in range(NT):
        if nb[t].sum() == 0:
            nb[t, 0] = 1
    return per_core, nb


def _prep_core(core_data, dinv, nb, NT):
    tile, half, vrel, dloc, s = core_data
    idx_cols, dl_cols, cf_cols = [], [], []
    chunks = []
    g = 0
    for c0 in range(0, NT, TILES_PER_CHUNK):
        tsel = list(range(c0, min(c0 + TILES_PER_CHUNK, NT)))
        ch = {"g0": g, "nb": [0, 0], "tiles": {t: [] for t in tsel}}
        for h in (0, 1):
            for t in tsel:
                m = (tile == t) & (half == h)
                vr, dl = vrel[m], dloc[m]
                cf = dinv[s[m]].astype(np.float32)
                want = nb[t, h] * 128
                pad = want - vr.shape[0]
                vr = np.concatenate([vr, np.zeros(pad, np.int64)])
                dl = np.concatenate([dl, np.zeros(pad, np.int64)])
                cf = np.concatenate([cf, np.zeros(pad, np.float32)])
                for b in range(nb[t, h]):
                    ch["tiles"][t].append(g)
                    idx_cols.append(vr[b * 128:(b + 1) * 128])
                    dl_cols.append(dl[b * 128:(b + 1) * 128])
                    cf_cols.append(cf[b * 128:(b + 1) * 128])
                    ch["nb"][h] += 1
                    g += 1
        chunks.append(ch)
    idx_all = np.concatenate(idx_cols).astype(np.int16)
    dl_all = np.stack(dl_cols, axis=1).astype(np.float32)
    cf_all = np.stack(cf_cols, axis=1).astype(np.float32)
    return _wrap_idx(idx_all), dl_all, cf_all, chunks, g


# ------------------------------------------------------------------ programs
def _build_layer_program(meta, lay):
    """One GCN layer as an SPMD program.  lay 0: BN-stats + L1; lay 1: L2;
    lay 2: L3 + pooled partials."""
    NPAD, SHARD, NT, JT, G, F, H, H2, H4, C, NB, N_true = (
        meta["NPAD"], meta["SHARD"], meta["NT"], meta["JT"], meta["G"],
        meta["F"], meta["H"], meta["H2"], meta["H4"], meta["C"],
        meta["NB"], meta["N_true"])
    chunks = meta["chunks"]
    STAT_CHUNK = 28
    HALF_ROWS = (NPAD // 128) * 64
    Ho = H if lay < 2 else H2

    nc = bacc.Bacc("TRN2", target_bir_lowering=False, debug=False,
                   num_devices=N_CORES)

    def din(name, shape, dt):
        return nc.dram_tensor(name, list(shape), dt, kind="ExternalInput").ap()

    x_d = din("h_in", [NPAD, F], BF16)
    idx_d = din("idx", [128, NB * 8], I16)
    dl_d = din("dstloc", [128, NB], F32)
    cf_d = din("coef", [128, NB], F32)
    iota_d = din("iota128", [128, 128], BF16)
    dinv_d = din("dinv_cols", [128, NT], F32)
    invd_d = din("invd_rows", [1, SHARD], BF16)
    w_d = din("W", [F, Ho], F32 if lay == 0 else BF16)
    b_d = din("br", [1, Ho], BF16)
    if lay == 0:
        ident_d = din("ident", [128, 128], F32)
        d2_d = din("d2_rows", [1, SHARD], BF16)
        gam_d = din("gamma", [128, 1], F32)
        bet_d = din("beta", [128, 1], F32)
    if lay == 2:
        iotag_d = din("iota_g", [128, G], BF16)
        bat_d = din("bat_cols", [128, NT], F32)
        ivc_d = din("ivc_cols", [128, NT], F32)
        pool_out = nc.dram_tensor("pool_part", [H2, G], F32,
                                  kind="ExternalOutput").ap()
    else:
        h_out = nc.dram_tensor("h_out", [SHARD, F], BF16,
                               kind="ExternalOutput").ap()

    with TileContext(nc) as tc:
        with contextlib.ExitStack() as ctx:
            cpool = ctx.enter_context(tc.tile_pool(name="const", bufs=1))

            def cload(name, shape, dt, src):
                t = cpool.tile(list(shape), dt, tag=name)
                nc.sync.dma_start(out=t[:], in_=src)
                return t

            iota_s = cload("c_iota", [128, 128], BF16, iota_d[:])
            idx_s = cload("c_idx", [128, NB * 8], I16, idx_d[:])
            dl_s = cload("c_dl", [128, NB], F32, dl_d[:])
            cf_s = cload("c_cf", [128, NB], F32, cf_d[:])
            dinv_s = cload("c_dinv", [128, NT], F32, dinv_d[:])
            invd_s = cload("c_invd", [1, SHARD], BF16, invd_d[:])
            b_s = cload("c_b", [1, Ho], BF16, b_d[:])
            if lay == 0:
                w1f_s = cload("c_w1f", [F, H], F32, w_d[:])
                ident_s = cload("c_ident", [128, 128], F32, ident_d[:])
                d2_s = cload("c_d2", [1, SHARD], BF16, d2_d[:])
                gam_s = cload("c_gam", [128, 1], F32, gam_d[:])
                bet_s = cload("c_bet", [128, 1], F32, bet_d[:])
                w_s = cpool.tile([F, H], BF16, tag="c_wt")
                rw_s = cpool.tile([1, H], BF16, tag="c_rw")
                ones_s = cpool.tile([128, 1], BF16, tag="c_ones")
                nc.vector.memset(ones_s[:], 1.0)
            else:
                w_s = cload("c_w", [F, Ho], BF16, w_d[:])
            if lay == 2:
                iotag_s = cload("c_iotag", [128, G], BF16, iotag_d[:])
                bat_s = cload("c_bat", [128, NT], F32, bat_d[:])
                ivc_s = cload("c_ivc", [128, NT], F32, ivc_d[:])

            x_t = x_d.rearrange("(p j) f -> p (j f)", p=128)

            # ---- BN statistics (layer 0 only) -> W~1 and shift row rw
            if lay == 0:
                with tc.tile_pool(name="ps_st", bufs=1, space="PSUM") as pst, \
                     tc.tile_pool(name="st_w", bufs=2) as stw:
                    xtx_ps = pst.tile([128, 128], F32, tag="xtx")
                    sx_ps = pst.tile([128, 1], F32, tag="sx")
                    nstat = (JT + STAT_CHUNK - 1) // STAT_CHUNK
                    for si in range(nstat):
                        j0, j1 = si * STAT_CHUNK, min((si + 1) * STAT_CHUNK, JT)
                        xt = stw.tile([128, STAT_CHUNK * F], BF16, tag="xt")
                        nc.sync.dma_start(out=xt[:, :(j1 - j0) * F],
                                          in_=x_t[:, j0 * F:j1 * F])
                        for j in range(j1 - j0):
                            sl = xt[:, j * F:(j + 1) * F]
                            st = (si == 0 and j == 0)
                            sp_ = (si == nstat - 1 and j == j1 - j0 - 1)
                            nc.tensor.matmul(xtx_ps[:], sl, sl,
                                             start=st, stop=sp_)
                            nc.tensor.matmul(sx_ps[:], sl, ones_s[:],
                                             start=st, stop=sp_)
                    dg = stw.tile([128, 128], F32, tag="dg")
                    nc.vector.tensor_tensor(dg[:], xtx_ps[:], ident_s[:],
                                            mybir.AluOpType.mult)
                    ex2 = stw.tile([128, 1], F32, tag="v1")
                    nc.vector.tensor_reduce(ex2[:], dg[:],
                                            mybir.AxisListType.X,
                                            mybir.AluOpType.add)
                    mu = stw.tile([128, 1], F32, tag="v2")
                    nc.vector.tensor_scalar_mul(mu[:], sx_ps[:], 1.0 / N_true)
                    var = stw.tile([128, 1], F32, tag="v3")
                    nc.vector.tensor_scalar_mul(var[:], ex2[:], 1.0 / N_true)
                    mu2 = stw.tile([128, 1], F32, tag="v4")
                    nc.vector.tensor_tensor(mu2[:], mu[:], mu[:],
                                            mybir.AluOpType.mult)
                    nc.vector.tensor_tensor(var[:], var[:], mu2[:],
                                            mybir.AluOpType.subtract)
                    nc.vector.tensor_scalar_add(var[:], var[:], BN_EPS)
                    rec = stw.tile([128, 1], F32, tag="v5")
                    nc.vector.reciprocal(rec[:], var[:])
                    isd = stw.tile([128, 1], F32, tag="v6")
                    nc.scalar.activation(isd[:], rec[:],
                                         mybir.ActivationFunctionType.Sqrt)
                    a_c = stw.tile([128, 1], F32, tag="v7")
                    nc.vector.tensor_tensor(a_c[:], gam_s[:], isd[:],
                                            mybir.AluOpType.mult)
                    nc.vector.tensor_scalar_mul(w_s[:], w1f_s[:], a_c[:])
                    ca = stw.tile([128, 1], F32, tag="v8")
                    nc.vector.tensor_tensor(ca[:], mu[:], a_c[:],
                                            mybir.AluOpType.mult)
                    nc.vector.tensor_tensor(ca[:], bet_s[:], ca[:],
                                            mybir.AluOpType.subtract)
                    rw_ps = pst.tile([1, H], F32, tag="rw")
                    nc.tensor.matmul(rw_ps[:], ca[:], w1f_s[:],
                                     start=True, stop=True)
                    nc.scalar.activation(rw_s[:], rw_ps[:],
                                         mybir.ActivationFunctionType.Copy)

            # ---- the layer itself
            lay_pool = ctx.enter_context(tc.tile_pool(name="lay", bufs=2))
            sp_pool = ctx.enter_context(tc.tile_pool(name="sp", bufs=4))
            ps_agg = ctx.enter_context(
                tc.tile_pool(name="ps_agg", bufs=2, space="PSUM"))
            ps_out = ctx.enter_context(
                tc.tile_pool(name="ps_out", bufs=2, space="PSUM"))
            if lay == 2:
                ps_pl = ctx.enter_context(
                    tc.tile_pool(name="ps_pl", bufs=1, space="PSUM"))
                pool_ps = ps_pl.tile([H2, G], F32, tag="pool")

            for ch in chunks:
                nb0, nb1 = ch["nb"]
                cb = nb0 + nb1
                g0 = ch["g0"]
                hg = lay_pool.tile([128, max(cb, 1), F], BF16, tag="hg")
                if nb0:
                    nc.gpsimd.dma_gather(
                        hg[:, :nb0, :], x_d[0:HALF_ROWS, :],
                        idx_s[:, g0 * 8:(g0 + nb0) * 8],
                        nb0 * 128, nb0 * 128, F, single_packet=False)
                if nb1:
                    nc.gpsimd.dma_gather(
                        hg[:, nb0:cb, :], x_d[HALF_ROWS:NPAD, :],
                        idx_s[:, (g0 + nb0) * 8:(g0 + cb) * 8],
                        nb1 * 128, nb1 * 128, F, single_packet=False)
                for t, blocks in ch["tiles"].items():
                    agg_ps = ps_agg.tile([128, 128], F32, tag="agg")
                    for bi, g in enumerate(blocks):
                        sp = sp_pool.tile([128, 128], BF16, tag="sp")
                        nc.vector.tensor_scalar(
                            sp[:], iota_s[:], dl_s[:, g:g + 1],
                            cf_s[:, g:g + 1],
                            mybir.AluOpType.is_equal, mybir.AluOpType.mult)
                        nc.tensor.matmul(
                            agg_ps[:], hg[:, g - g0, :], sp[:],
                            start=(bi == 0), stop=(bi == len(blocks) - 1))
                    aggT = sp_pool.tile([128, 128], BF16, tag="aggT")
                    nc.scalar.activation(aggT[:], agg_ps[:],
                                         mybir.ActivationFunctionType.Copy)
                    h_ps = ps_out.tile([128, Ho], F32, tag="hps")
                    nc.tensor.matmul(h_ps[:], aggT[:], w_s[:],
                                     start=True, stop=False)
                    if lay == 0:
                        nc.tensor.matmul(
                            h_ps[:], d2_s[0:1, t * 128:(t + 1) * 128],
                            rw_s[:], start=False, stop=False)
                    nc.tensor.matmul(
                        h_ps[:], invd_s[0:1, t * 128:(t + 1) * 128],
                        b_s[:], start=False, stop=True)
                    hs = sp_pool.tile([128, Ho], BF16, tag="hs")
                    nc.scalar.activation(hs[:], h_ps[:],
                                         mybir.ActivationFunctionType.Relu,
                                         scale=dinv_s[:, t:t + 1])
                    if lay < 2:
                        nc.sync.dma_start(
                            out=h_out[t * 128:(t + 1) * 128, :], in_=hs[:])
                    else:
                        g1 = sp_pool.tile([128, G], BF16, tag="g1")
                        nc.vector.tensor_scalar(
                            g1[:], iotag_s[:], bat_s[:, t:t + 1],
                            ivc_s[:, t:t + 1],
                            mybir.AluOpType.is_equal, mybir.AluOpType.mult)
                        nc.tensor.matmul(pool_ps[:], hs[:], g1[:],
                                         start=(t == 0), stop=(t == NT - 1),
                                         skip_group_check=True)
            if lay == 2:
                po = sp_pool.tile([H2, G], F32, tag="po")
                nc.vector.tensor_copy(po[:], pool_ps[:])
                nc.sync.dma_start(out=pool_out[:], in_=po[:])

    nc.compile()
    return nc


def _build_mlp_program(meta):
    G, H2, H4, C = meta["G"], meta["H2"], meta["H4"], meta["C"]
    nc = bacc.Bacc("TRN2", target_bir_lowering=False, debug=False,
                   num_devices=N_CORES)
    pp_d = nc.dram_tensor("pool_parts", [N_CORES * H2, G], F32,
                          kind="ExternalInput").ap()
    wc1_d = nc.dram_tensor("Wc1", [H2, H4], F32, kind="ExternalInput").ap()
    wc2_d = nc.dram_tensor("Wc2", [H4, C], F32, kind="ExternalInput").ap()
    bc1_d = nc.dram_tensor("bc1", [H4, 1], F32, kind="ExternalInput").ap()
    bc2_d = nc.dram_tensor("bc2b", [G, C], F32, kind="ExternalInput").ap()
    out_d = nc.dram_tensor("out", [G, C], F32, kind="ExternalOutput").ap()

    with TileContext(nc) as tc:
        with tc.tile_pool(name="w", bufs=1) as wp, \
             tc.tile_pool(name="ps", bufs=1, space="PSUM") as pp:
            wc1_s = wp.tile([H2, H4], F32)
            nc.sync.dma_start(out=wc1_s[:], in_=wc1_d[:])
            wc2_s = wp.tile([H4, C], F32)
            nc.sync.dma_start(out=wc2_s[:], in_=wc2_d[:])
            bc1_s = wp.tile([H4, 1], F32)
            nc.sync.dma_start(out=bc1_s[:], in_=bc1_d[:])
            bc2_s = wp.tile([G, C], F32)
            nc.sync.dma_start(out=bc2_s[:], in_=bc2_d[:])
            acc = wp.tile([H2, G], F32)
            t0 = wp.tile([H2, G], F32, tag="t0")
            nc.sync.dma_start(out=acc[:], in_=pp_d[0:H2, :])
            for s in range(1, N_CORES):
                ts = wp.tile([H2, G], F32, tag=f"t{s}")
                nc.sync.dma_start(out=ts[:], in_=pp_d[s * H2:(s + 1) * H2, :])
                nc.vector.tensor_tensor(acc[:], acc[:], ts[:],
                                        mybir.AluOpType.add)
            z_ps = pp.tile([H4, G], F32, tag="z")
            nc.tensor.matmul(z_ps[:], wc1_s[:], acc[:], start=True, stop=True)
            z_s = wp.tile([H4, G], F32, tag="zs")
            nc.scalar.activation(z_s[:], z_ps[:],
                                 mybir.ActivationFunctionType.Relu,
                                 bias=bc1_s[:])
            o_ps = pp.tile([G, C], F32, tag="o")
            nc.tensor.matmul(o_ps[:], z_s[:], wc2_s[:], start=True, stop=True)
            o_s = wp.tile([G, C], F32, tag="os")
            nc.vector.tensor_tensor(o_s[:], o_ps[:], bc2_s[:],
                                    mybir.AluOpType.add)
            nc.sync.dma_start(out=out_d[:], in_=o_s[:])
    nc.compile()
    return nc


# ------------------------------------------------------------------ driver
def _prep_inputs(inputs, NT):
    x = np.asarray(inputs["x"], np.float32)
    N_true, F = x.shape
    W1 = np.asarray(inputs["W1"], np.float32)
    W2 = np.asarray(inputs["W2"], np.float32)
    W3 = np.asarray(inputs["W3"], np.float32)
    Wc1 = np.asarray(inputs["Wc1"], np.float32)
    Wc2 = np.asarray(inputs["Wc2"], np.float32)
    H, H2, H4, C = W1.shape[1], W3.shape[1], Wc1.shape[1], Wc2.shape[1]
    G = 64
    SHARD = NT * 128
    NPAD = N_CORES * SHARD
    JT = NPAD // 128

    src = np.asarray(inputs["edge_index"][0], np.int64)
    dst = np.asarray(inputs["edge_index"][1], np.int64)
    batch = np.asarray(inputs["batch"], np.int64)

    deg = np.bincount(dst, minlength=N_true).astype(np.float64) + 1.0
    dinv_t = (1.0 / np.sqrt(deg)).astype(np.float32)
    dinv = np.ones(NPAD, np.float32)
    dinv[:N_true] = dinv_t
    selfv = np.arange(N_true, dtype=np.int64)
    src_a = np.concatenate([src, selfv])
    dst_a = np.concatenate([dst, selfv])
    sneig = np.bincount(dst, weights=dinv_t[src].astype(np.float64),
                        minlength=N_true)
    # outer dinv[dst] is applied by the ReLU's per-partition scale on device
    d2_t = (sneig + dinv_t).astype(np.float32)

    per_core, nb = _common_schedule(src_a, dst_a, NT, SHARD, JT)
    core_edge, NB, chunks = [], None, None
    for c in range(N_CORES):
        idx_t, dl_t, cf_t, ch, nbt = _prep_core(per_core[c], dinv, nb, NT)
        NB = nbt
        chunks = ch
        core_edge.append((idx_t, dl_t, cf_t))

    perm = (np.arange(NPAD) % 128) * JT + (np.arange(NPAD) // 128)

    def tile_major(h_nodemajor):  # [NPAD, F] node-major -> row-permuted
        out = np.empty_like(h_nodemajor)
        out[perm] = h_nodemajor
        return out

    xp = np.zeros((NPAD, F), ml_dtypes.bfloat16)
    xp[:N_true] = x.astype(ml_dtypes.bfloat16)
    x_tl = tile_major(xp)

    iota128 = np.tile(np.arange(128, dtype=ml_dtypes.bfloat16)[None, :],
                      (128, 1)).copy()
    iota_g = np.tile(np.arange(G, dtype=ml_dtypes.bfloat16)[None, :],
                     (128, 1)).copy()

    def cols(vec, fill):
        v = np.full(NPAD, fill, np.float32)
        v[:N_true] = vec
        return v.reshape(N_CORES, NT, 128).transpose(0, 2, 1).copy()

    dinv_cols = cols(dinv_t, 1.0)
    cnt = np.bincount(batch, minlength=G).astype(np.float64)
    invc = (1.0 / np.maximum(cnt, 1.0)).astype(np.float32)
    bat_cols = cols(batch.astype(np.float32), 999.0)
    ivc_cols = cols(invc[batch], 0.0)

    def rows(vec, fill=0.0, dt=ml_dtypes.bfloat16):
        v = np.full(NPAD, fill, np.float32)
        v[:N_true] = vec
        return v.reshape(N_CORES, 1, SHARD).astype(dt)

    d2_rows = rows(d2_t)
    invd_rows = rows(np.sqrt(deg).astype(np.float32))

    meta = {"NPAD": NPAD, "SHARD": SHARD, "NT": NT, "JT": JT, "G": G,
            "F": F, "H": H, "H2": H2, "H4": H4, "C": C, "NB": NB,
            "N_true": N_true, "chunks": chunks}

    bf = ml_dtypes.bfloat16
    stat = {
        "iota128": iota128, "iota_g": iota_g,
        "ident": np.eye(128, dtype=np.float32),
        "gamma": np.asarray(inputs["bn_gamma"], np.float32).reshape(F, 1),
        "beta": np.asarray(inputs["bn_beta"], np.float32).reshape(F, 1),
        "W": [W1, W2.astype(bf), W3.astype(bf)],
        "br": [np.asarray(inputs["b1"], np.float32).reshape(1, H).astype(bf),
               np.asarray(inputs["b2"], np.float32).reshape(1, H).astype(bf),
               np.asarray(inputs["b3"], np.float32).reshape(1, H2).astype(bf)],
        "Wc1": Wc1, "Wc2": Wc2,
        "bc1": np.asarray(inputs["bc1"], np.float32).reshape(H4, 1),
        "bc2b": np.tile(np.asarray(inputs["bc2"], np.float32)[None, :],
                        (G, 1)).copy(),
        "x_tl": x_tl, "tile_major": tile_major,
        "dinv_cols": dinv_cols, "bat_cols": bat_cols, "ivc_cols": ivc_cols,
        "d2_rows": d2_rows, "invd_rows": invd_rows, "core_edge": core_edge,
    }
    return meta, stat


_CACHE = {}


def _get_programs(meta):
    key = (meta["NT"], meta["NB"], str(meta["chunks"]))
    if key not in _CACHE:
        progs = [_build_layer_program(meta, lay) for lay in range(3)]
        progs.append(_build_mlp_program(meta))
        _CACHE[key] = progs
    return _CACHE[key]


def run_gnn(NT=49, runner=None, **inputs):
    meta, st = _prep_inputs(inputs, NT)
    SHARD, NPAD, F = meta["SHARD"], meta["NPAD"], meta["F"]
    progs = _get_programs(meta)

    def run(nc, in_maps):
        if runner is not None:
            return runner(nc, in_maps)
        return run_bass_kernel_spmd(
            nc, in_maps, core_ids=list(range(N_CORES))).results

    def layer_maps(lay, h_in):
        maps = []
        for c in range(N_CORES):
            idx_t, dl_t, cf_t = st["core_edge"][c]
            m = {"h_in": h_in, "idx": idx_t, "dstloc": dl_t, "coef": cf_t,
                 "iota128": st["iota128"], "dinv_cols": st["dinv_cols"][c],
                 "invd_rows": st["invd_rows"][c], "W": st["W"][lay],
                 "br": st["br"][lay]}
            if lay == 0:
                m.update(ident=st["ident"], d2_rows=st["d2_rows"][c],
                         gamma=st["gamma"], beta=st["beta"])
            if lay == 2:
                m.update(iota_g=st["iota_g"], bat_cols=st["bat_cols"][c],
                         ivc_cols=st["ivc_cols"][c])
            maps.append(m)
        return maps

    h = st["x_tl"]
    for lay in range(2):
        res = run(progs[lay], layer_maps(lay, h))
        shards = [np.asarray(res[c]["h_out"]) for c in range(N_CORES)]
        h = st["tile_major"](np.concatenate(shards, axis=0))
    res = run(progs[2], layer_maps(2, h))
    pool_parts = np.concatenate(
        [np.asarray(res[c]["pool_part"]) for c in range(N_CORES)], axis=0)
    mlp_map = {"pool_parts": pool_parts, "Wc1": st["Wc1"], "Wc2": st["Wc2"],
               "bc1": st["bc1"], "bc2b": st["bc2b"]}
    res = run(progs[3], [dict(mlp_map) for _ in range(N_CORES)])
    return np.asarray(res[0]["out"], np.float32)


def kernel(**inputs):
    return run_gnn(NT=49, **inputs)


# revision 11
# speedup vs baseline: 1.0758x; 1.0758x over previous
"""Trainium2 Bass kernel for AudioOnlyGNN (3-layer GCN + BatchNorm + mean-pool + MLP).

Sharding: nodes padded to NPAD = 8*NT*128, split contiguously across the 8
NeuronCores; each core owns edges whose dst lands in its shard.  Per layer:
  - h rows gathered from HBM with dma_gather (bf16, 128-edge blocks, indices
    precomputed/permuted on host to a tile-major row order),
  - one-hot "S panels" (dst one-hot * dinv[src]) built on VectorE with one
    tensor_scalar(is_equal, mult) vs a constant iota tile,
  - TensorE matmul accumulates [F, dst-tile] PSUM tiles (contraction = edges),
  - [dst,H] = aggT @ W on TensorE + rank-1 bias matmuls,
  - ReLU with per-partition dinv[dst] scale on ScalarE.
BatchNorm is folded into W1 plus a rank-1 shift term from on-device batch
statistics (sum / sum-of-squares via matmuls).  Mean-pool uses the same
one-hot trick per dst tile.  The 3 GCN layers run as 3 SPMD launches; between
launches the host only concatenates/permutes the 8 output shards (pure data
movement) to form the replicated full-h input of the next launch.  A tiny 4th
launch sums the per-core pooled partials and runs the classifier MLP.
"""

import sys

sys.path.insert(0, "/opt/trn_rl_repo")

import contextlib

import numpy as np
import ml_dtypes

import concourse.bacc as bacc
import concourse.bass as bass
import concourse.mybir as mybir
from concourse.tile import TileContext
from concourse.bass_utils import run_bass_kernel_spmd

BF16 = mybir.dt.bfloat16
F32 = mybir.dt.float32
I16 = mybir.dt.int16

N_CORES = 8
BN_EPS = 1e-5
TILES_PER_CHUNK = 2


# ------------------------------------------------------------------ host prep
def _wrap_idx(idx):
    n = idx.shape[0]
    assert n % 16 == 0
    a = idx.astype(np.int16).reshape(n // 16, 16).T  # [16, n/16]
    return np.tile(a, (8, 1)).copy()


def _common_schedule(src, dst, NT, SHARD, JT):
    """Per-core edge lists; per-(tile, half) block counts maxed over cores so
    the SPMD program is identical on every core."""
    per_core = []
    for c in range(N_CORES):
        base = c * SHARD
        sel = (dst >= base) & (dst < base + SHARD)
        s, d = src[sel], dst[sel]
        tile = (d - base) // 128
        dloc = (d - base) % 128
        half = ((s % 128) >= 64).astype(np.int64)
        vperm = (s % 128).astype(np.int64) * JT + (s // 128)
        vrel = np.where(half == 1, vperm - 64 * JT, vperm)
        per_core.append((tile, half, vrel, dloc, s))

    nb = np.zeros((NT, 2), np.int64)
    for tile, half, _, _, _ in per_core:
        for h in (0, 1):
            cnt = np.bincount(tile[half == h], minlength=NT)
            nb[:, h] = np.maximum(nb[:, h], (cnt + 127) // 128)
    for t in range(NT):
        if nb[t].sum() == 0:
            nb[t, 0] = 1
    return per_core, nb


def _prep_core(core_data, dinv, nb, NT):
    tile, half, vrel, dloc, s = core_data
    idx_cols, dl_cols, cf_cols = [], [], []
    chunks = []
    g = 0
    for c0 in range(0, NT, TILES_PER_CHUNK):
        tsel = list(range(c0, min(c0 + TILES_PER_CHUNK, NT)))
        ch = {"g0": g, "nb": [0, 0], "tiles": {t: [] for t in tsel}}
        for h in (0, 1):
            for t in tsel:
                m = (tile == t) & (half == h)
                vr, dl = vrel[m], dloc[m]
                cf = dinv[s[m]].astype(np.float32)
                want = nb[t, h] * 128
                pad = want - vr.shape[0]
                vr = np.concatenate([vr, np.zeros(pad, np.int64)])
                dl = np.concatenate([dl, np.zeros(pad, np.int64)])
                cf = np.concatenate([cf, np.zeros(pad, np.float32)])
                for b in range(nb[t, h]):
                    ch["tiles"][t].append(g)
                    idx_cols.append(vr[b * 128:(b + 1) * 128])
                    dl_cols.append(dl[b * 128:(b + 1) * 128])
                    cf_cols.append(cf[b * 128:(b + 1) * 128])
                    ch["nb"][h] += 1
                    g += 1
        chunks.append(ch)
    idx_all = np.concatenate(idx_cols).astype(np.int16)
    dl_all = np.stack(dl_cols, axis=1).astype(np.float32)
    cf_all = np.stack(cf_cols, axis=1).astype(np.float32)
    return _wrap_idx(idx_all), dl_all, cf_all, chunks, g


# ------------------------------------------------------------------ programs
def _build_layer_program(meta, lay):
    """One GCN layer as an SPMD program.  lay 0: BN-stats + L1; lay 1: L2;
    lay 2: L3 + pooled partials."""
    NPAD, SHARD, NT, JT, G, F, H, H2, H4, C, NB, N_true = (
        meta["NPAD"], meta["SHARD"], meta["NT"], meta["JT"], meta["G"],
        meta["F"], meta["H"], meta["H2"], meta["H4"], meta["C"],
        meta["NB"], meta["N_true"])
    chunks = meta["chunks"]
    STAT_CHUNK = 49
    HALF_ROWS = (NPAD // 128) * 64
    Ho = H if lay < 2 else H2

    nc = bacc.Bacc("TRN2", target_bir_lowering=False, debug=False,
                   num_devices=N_CORES)

    def din(name, shape, dt):
        return nc.dram_tensor(name, list(shape), dt, kind="ExternalInput").ap()

    x_d = din("h_in", [NPAD, F], BF16)
    idx_d = din("idx", [128, NB * 8], I16)
    dl_d = din("dstloc", [128, NB], F32)
    cf_d = din("coef", [128, NB], F32)
    iota_d = din("iota128", [128, 128], BF16)
    dinv_d = din("dinv_cols", [128, NT], F32)
    invd_d = din("invd_rows", [1, SHARD], BF16)
    w_d = din("W", [F, Ho], F32 if lay == 0 else BF16)
    b_d = din("br", [1, Ho], BF16)
    if lay == 0:
        ident_d = din("ident", [128, 128], F32)
        d2_d = din("d2_rows", [1, SHARD], BF16)
        gam_d = din("gamma", [128, 1], F32)
        bet_d = din("beta", [128, 1], F32)
    if lay == 2:
        iotag_d = din("iota_g", [128, G], BF16)
        bat_d = din("bat_cols", [128, NT], F32)
        ivc_d = din("ivc_cols", [128, NT], F32)
        pool_out = nc.dram_tensor("pool_part", [H2, G], F32,
                                  kind="ExternalOutput").ap()
    else:
        h_out = nc.dram_tensor("h_out", [SHARD, F], BF16,
                               kind="ExternalOutput").ap()

    with TileContext(nc) as tc:
        with contextlib.ExitStack() as ctx:
            cpool = ctx.enter_context(tc.tile_pool(name="const", bufs=1))

            def cload(name, shape, dt, src):
                t = cpool.tile(list(shape), dt, tag=name)
                nc.sync.dma_start(out=t[:], in_=src)
                return t

            iota_s = cload("c_iota", [128, 128], BF16, iota_d[:])
            idx_s = cload("c_idx", [128, NB * 8], I16, idx_d[:])
            dl_s = cload("c_dl", [128, NB], F32, dl_d[:])
            cf_s = cload("c_cf", [128, NB], F32, cf_d[:])
            dinv_s = cload("c_dinv", [128, NT], F32, dinv_d[:])
            invd_s = cload("c_invd", [1, SHARD], BF16, invd_d[:])
            b_s = cload("c_b", [1, Ho], BF16, b_d[:])
            if lay == 0:
                w1f_s = cload("c_w1f", [F, H], F32, w_d[:])
                ident_s = cload("c_ident", [128, 128], F32, ident_d[:])
                d2_s = cload("c_d2", [1, SHARD], BF16, d2_d[:])
                gam_s = cload("c_gam", [128, 1], F32, gam_d[:])
                bet_s = cload("c_bet", [128, 1], F32, bet_d[:])
                w_s = cpool.tile([F, H], BF16, tag="c_wt")
                rw_s = cpool.tile([1, H], BF16, tag="c_rw")
                ones_s = cpool.tile([128, 1], BF16, tag="c_ones")
                nc.vector.memset(ones_s[:], 1.0)
            else:
                w_s = cload("c_w", [F, Ho], BF16, w_d[:])
            if lay == 2:
                iotag_s = cload("c_iotag", [128, G], BF16, iotag_d[:])
                bat_s = cload("c_bat", [128, NT], F32, bat_d[:])
                ivc_s = cload("c_ivc", [128, NT], F32, ivc_d[:])

            x_t = x_d.rearrange("(p j) f -> p (j f)", p=128)

            # ---- BN statistics (layer 0 only) -> W~1 and shift row rw
            if lay == 0:
                with tc.tile_pool(name="ps_st", bufs=1, space="PSUM") as pst, \
                     tc.tile_pool(name="st_w", bufs=2) as stw:
                    xtx_ps = pst.tile([128, 128], F32, tag="xtx")
                    sx_ps = pst.tile([128, 1], F32, tag="sx")
                    nstat = (JT + STAT_CHUNK - 1) // STAT_CHUNK
                    for si in range(nstat):
                        j0, j1 = si * STAT_CHUNK, min((si + 1) * STAT_CHUNK, JT)
                        xt = stw.tile([128, STAT_CHUNK * F], BF16, tag="xt")
                        nc.sync.dma_start(out=xt[:, :(j1 - j0) * F],
                                          in_=x_t[:, j0 * F:j1 * F])
                        for j in range(j1 - j0):
                            sl = xt[:, j * F:(j + 1) * F]
                            st = (si == 0 and j == 0)
                            sp_ = (si == nstat - 1 and j == j1 - j0 - 1)
                            nc.tensor.matmul(xtx_ps[:], sl, sl,
                                             start=st, stop=sp_)
                            nc.tensor.matmul(sx_ps[:], sl, ones_s[:],
                                             start=st, stop=sp_)
                    dg = stw.tile([128, 128], F32, tag="dg")
                    nc.vector.tensor_tensor(dg[:], xtx_ps[:], ident_s[:],
                                            mybir.AluOpType.mult)
                    ex2 = stw.tile([128, 1], F32, tag="v1")
                    nc.vector.tensor_reduce(ex2[:], dg[:],
                                            mybir.AxisListType.X,
                                            mybir.AluOpType.add)
                    mu = stw.tile([128, 1], F32, tag="v2")
                    nc.vector.tensor_scalar_mul(mu[:], sx_ps[:], 1.0 / N_true)
                    var = stw.tile([128, 1], F32, tag="v3")
                    nc.vector.tensor_scalar_mul(var[:], ex2[:], 1.0 / N_true)
                    mu2 = stw.tile([128, 1], F32, tag="v4")
                    nc.vector.tensor_tensor(mu2[:], mu[:], mu[:],
                                            mybir.AluOpType.mult)
                    nc.vector.tensor_tensor(var[:], var[:], mu2[:],
                                            mybir.AluOpType.subtract)
                    nc.vector.tensor_scalar_add(var[:], var[:], BN_EPS)
                    rec = stw.tile([128, 1], F32, tag="v5")
                    nc.vector.reciprocal(rec[:], var[:])
                    isd = stw.tile([128, 1], F32, tag="v6")
                    nc.scalar.activation(isd[:], rec[:],
                                         mybir.ActivationFunctionType.Sqrt)
                    a_c = stw.tile([128, 1], F32, tag="v7")
                    nc.vector.tensor_tensor(a_c[:], gam_s[:], isd[:],
                                            mybir.AluOpType.mult)
                    nc.vector.tensor_scalar_mul(w_s[:], w1f_s[:], a_c[:])
                    ca = stw.tile([128, 1], F32, tag="v8")
                    nc.vector.tensor_tensor(ca[:], mu[:], a_c[:],
                                            mybir.AluOpType.mult)
                    nc.vector.tensor_tensor(ca[:], bet_s[:], ca[:],
                                            mybir.AluOpType.subtract)
                    rw_ps = pst.tile([1, H], F32, tag="rw")
                    nc.tensor.matmul(rw_ps[:], ca[:], w1f_s[:],
                                     start=True, stop=True)
                    nc.scalar.activation(rw_s[:], rw_ps[:],
                                         mybir.ActivationFunctionType.Copy)

            # ---- the layer itself
            lay_pool = ctx.enter_context(tc.tile_pool(name="lay", bufs=3))
            sp_pool = ctx.enter_context(tc.tile_pool(name="sp", bufs=4))
            ps_agg = ctx.enter_context(
                tc.tile_pool(name="ps_agg", bufs=2, space="PSUM"))
            ps_out = ctx.enter_context(
                tc.tile_pool(name="ps_out", bufs=2, space="PSUM"))
            if lay == 2:
                ps_pl = ctx.enter_context(
                    tc.tile_pool(name="ps_pl", bufs=1, space="PSUM"))
                pool_ps = ps_pl.tile([H2, G], F32, tag="pool")

            for ch in chunks:
                nb0, nb1 = ch["nb"]
                cb = nb0 + nb1
                g0 = ch["g0"]
                hg = lay_pool.tile([128, max(cb, 1), F], BF16, tag="hg")
                if nb0:
                    nc.gpsimd.dma_gather(
                        hg[:, :nb0, :], x_d[0:HALF_ROWS, :],
                        idx_s[:, g0 * 8:(g0 + nb0) * 8],
                        nb0 * 128, nb0 * 128, F, single_packet=False)
                if nb1:
                    nc.gpsimd.dma_gather(
                        hg[:, nb0:cb, :], x_d[HALF_ROWS:NPAD, :],
                        idx_s[:, (g0 + nb0) * 8:(g0 + cb) * 8],
                        nb1 * 128, nb1 * 128, F, single_packet=False)
                for t, blocks in ch["tiles"].items():
                    agg_ps = ps_agg.tile([128, 128], F32, tag="agg")
                    for bi, g in enumerate(blocks):
                        sp = sp_pool.tile([128, 128], BF16, tag="sp")
                        nc.vector.tensor_scalar(
                            sp[:], iota_s[:], dl_s[:, g:g + 1],
                            cf_s[:, g:g + 1],
                            mybir.AluOpType.is_equal, mybir.AluOpType.mult)
                        nc.tensor.matmul(
                            agg_ps[:], hg[:, g - g0, :], sp[:],
                            start=(bi == 0), stop=(bi == len(blocks) - 1))
                    aggT = sp_pool.tile([128, 128], BF16, tag="aggT")
                    nc.scalar.activation(aggT[:], agg_ps[:],
                                         mybir.ActivationFunctionType.Copy)
                    h_ps = ps_out.tile([128, Ho], F32, tag="hps")
                    nc.tensor.matmul(h_ps[:], aggT[:], w_s[:],
                                     start=True, stop=False)
                    if lay == 0:
                        nc.tensor.matmul(
                            h_ps[:], d2_s[0:1, t * 128:(t + 1) * 128],
                            rw_s[:], start=False, stop=False)
                    nc.tensor.matmul(
                        h_ps[:], invd_s[0:1, t * 128:(t + 1) * 128],
                        b_s[:], start=False, stop=True)
                    hs = sp_pool.tile([128, Ho], BF16, tag="hs")
                    nc.scalar.activation(hs[:], h_ps[:],
                                         mybir.ActivationFunctionType.Relu,
                                         scale=dinv_s[:, t:t + 1])
                    if lay < 2:
                        nc.sync.dma_start(
                            out=h_out[t * 128:(t + 1) * 128, :], in_=hs[:])
                    else:
                        g1 = sp_pool.tile([128, G], BF16, tag="g1")
                        nc.vector.tensor_scalar(
                            g1[:], iotag_s[:], bat_s[:, t:t + 1],
                            ivc_s[:, t:t + 1],
                            mybir.AluOpType.is_equal, mybir.AluOpType.mult)
                        nc.tensor.matmul(pool_ps[:], hs[:], g1[:],
                                         start=(t == 0), stop=(t == NT - 1),
                                         skip_group_check=True)
            if lay == 2:
                po = sp_pool.tile([H2, G], F32, tag="po")
                nc.vector.tensor_copy(po[:], pool_ps[:])
                nc.sync.dma_start(out=pool_out[:], in_=po[:])

    nc.compile()
    return nc


def _build_mlp_program(meta):
    G, H2, H4, C = meta["G"], meta["H2"], meta["H4"], meta["C"]
    nc = bacc.Bacc("TRN2", target_bir_lowering=False, debug=False,
                   num_devices=N_CORES)
    pp_d = nc.dram_tensor("pool_parts", [N_CORES * H2, G], F32,
                          kind="ExternalInput").ap()
    wc1_d = nc.dram_tensor("Wc1", [H2, H4], F32, kind="ExternalInput").ap()
    wc2_d = nc.dram_tensor("Wc2", [H4, C], F32, kind="ExternalInput").ap()
    bc1_d = nc.dram_tensor("bc1", [H4, 1], F32, kind="ExternalInput").ap()
    bc2_d = nc.dram_tensor("bc2b", [G, C], F32, kind="ExternalInput").ap()
    out_d = nc.dram_tensor("out", [G, C], F32, kind="ExternalOutput").ap()

    with TileContext(nc) as tc:
        with tc.tile_pool(name="w", bufs=1) as wp, \
             tc.tile_pool(name="ps", bufs=1, space="PSUM") as pp:
            wc1_s = wp.tile([H2, H4], F32)
            nc.sync.dma_start(out=wc1_s[:], in_=wc1_d[:])
            wc2_s = wp.tile([H4, C], F32)
            nc.sync.dma_start(out=wc2_s[:], in_=wc2_d[:])
            bc1_s = wp.tile([H4, 1], F32)
            nc.sync.dma_start(out=bc1_s[:], in_=bc1_d[:])
            bc2_s = wp.tile([G, C], F32)
            nc.sync.dma_start(out=bc2_s[:], in_=bc2_d[:])
            acc = wp.tile([H2, G], F32)
            t0 = wp.tile([H2, G], F32, tag="t0")
            nc.sync.dma_start(out=acc[:], in_=pp_d[0:H2, :])
            for s in range(1, N_CORES):
                ts = wp.tile([H2, G], F32, tag=f"t{s}")
                nc.sync.dma_start(out=ts[:], in_=pp_d[s * H2:(s + 1) * H2, :])
                nc.vector.tensor_tensor(acc[:], acc[:], ts[:],
                                        mybir.AluOpType.add)
            z_ps = pp.tile([H4, G], F32, tag="z")
            nc.tensor.matmul(z_ps[:], wc1_s[:], acc[:], start=True, stop=True)
            z_s = wp.tile([H4, G], F32, tag="zs")
            nc.scalar.activation(z_s[:], z_ps[:],
                                 mybir.ActivationFunctionType.Relu,
                                 bias=bc1_s[:])
            o_ps = pp.tile([G, C], F32, tag="o")
            nc.tensor.matmul(o_ps[:], z_s[:], wc2_s[:], start=True, stop=True)
            o_s = wp.tile([G, C], F32, tag="os")
            nc.vector.tensor_tensor(o_s[:], o_ps[:], bc2_s[:],
                                    mybir.AluOpType.add)
            nc.sync.dma_start(out=out_d[:], in_=o_s[:])
    nc.compile()
    return nc


# ------------------------------------------------------------------ driver
def _prep_inputs(inputs, NT):
    x = np.asarray(inputs["x"], np.float32)
    N_true, F = x.shape
    W1 = np.asarray(inputs["W1"], np.float32)
    W2 = np.asarray(inputs["W2"], np.float32)
    W3 = np.asarray(inputs["W3"], np.float32)
    Wc1 = np.asarray(inputs["Wc1"], np.float32)
    Wc2 = np.asarray(inputs["Wc2"], np.float32)
    H, H2, H4, C = W1.shape[1], W3.shape[1], Wc1.shape[1], Wc2.shape[1]
    G = 64
    SHARD = NT * 128
    NPAD = N_CORES * SHARD
    JT = NPAD // 128

    src = np.asarray(inputs["edge_index"][0], np.int64)
    dst = np.asarray(inputs["edge_index"][1], np.int64)
    batch = np.asarray(inputs["batch"], np.int64)

    deg = np.bincount(dst, minlength=N_true).astype(np.float64) + 1.0
    dinv_t = (1.0 / np.sqrt(deg)).astype(np.float32)
    dinv = np.ones(NPAD, np.float32)
    dinv[:N_true] = dinv_t
    selfv = np.arange(N_true, dtype=np.int64)
    src_a = np.concatenate([src, selfv])
    dst_a = np.concatenate([dst, selfv])
    sneig = np.bincount(dst, weights=dinv_t[src].astype(np.float64),
                        minlength=N_true)
    # outer dinv[dst] is applied by the ReLU's per-partition scale on device
    d2_t = (sneig + dinv_t).astype(np.float32)

    per_core, nb = _common_schedule(src_a, dst_a, NT, SHARD, JT)
    core_edge, NB, chunks = [], None, None
    for c in range(N_CORES):
        idx_t, dl_t, cf_t, ch, nbt = _prep_core(per_core[c], dinv, nb, NT)
        NB = nbt
        chunks = ch
        core_edge.append((idx_t, dl_t, cf_t))

    perm = (np.arange(NPAD) % 128) * JT + (np.arange(NPAD) // 128)

    def tile_major(h_nodemajor):  # [NPAD, F] node-major -> row-permuted
        out = np.empty_like(h_nodemajor)
        out[perm] = h_nodemajor
        return out

    xp = np.zeros((NPAD, F), ml_dtypes.bfloat16)
    xp[:N_true] = x.astype(ml_dtypes.bfloat16)
    x_tl = tile_major(xp)

    iota128 = np.tile(np.arange(128, dtype=ml_dtypes.bfloat16)[None, :],
                      (128, 1)).copy()
    iota_g = np.tile(np.arange(G, dtype=ml_dtypes.bfloat16)[None, :],
                     (128, 1)).copy()

    def cols(vec, fill):
        v = np.full(NPAD, fill, np.float32)
        v[:N_true] = vec
        return v.reshape(N_CORES, NT, 128).transpose(0, 2, 1).copy()

    dinv_cols = cols(dinv_t, 1.0)
    cnt = np.bincount(batch, minlength=G).astype(np.float64)
    invc = (1.0 / np.maximum(cnt, 1.0)).astype(np.float32)
    bat_cols = cols(batch.astype(np.float32), 999.0)
    ivc_cols = cols(invc[batch], 0.0)

    def rows(vec, fill=0.0, dt=ml_dtypes.bfloat16):
        v = np.full(NPAD, fill, np.float32)
        v[:N_true] = vec
        return v.reshape(N_CORES, 1, SHARD).astype(dt)

    d2_rows = rows(d2_t)
    invd_rows = rows(np.sqrt(deg).astype(np.float32))

    meta = {"NPAD": NPAD, "SHARD": SHARD, "NT": NT, "JT": JT, "G": G,
            "F": F, "H": H, "H2": H2, "H4": H4, "C": C, "NB": NB,
            "N_true": N_true, "chunks": chunks}

    bf = ml_dtypes.bfloat16
    stat = {
        "iota128": iota128, "iota_g": iota_g,
        "ident": np.eye(128, dtype=np.float32),
        "gamma": np.asarray(inputs["bn_gamma"], np.float32).reshape(F, 1),
        "beta": np.asarray(inputs["bn_beta"], np.float32).reshape(F, 1),
        "W": [W1, W2.astype(bf), W3.astype(bf)],
        "br": [np.asarray(inputs["b1"], np.float32).reshape(1, H).astype(bf),
               np.asarray(inputs["b2"], np.float32).reshape(1, H).astype(bf),
               np.asarray(inputs["b3"], np.float32).reshape(1, H2).astype(bf)],
        "Wc1": Wc1, "Wc2": Wc2,
        "bc1": np.asarray(inputs["bc1"], np.float32).reshape(H4, 1),
        "bc2b": np.tile(np.asarray(inputs["bc2"], np.float32)[None, :],
                        (G, 1)).copy(),
        "x_tl": x_tl, "tile_major": tile_major,
        "dinv_cols": dinv_cols, "bat_cols": bat_cols, "ivc_cols": ivc_cols,
        "d2_rows": d2_rows, "invd_rows": invd_rows, "core_edge": core_edge,
    }
    return meta, stat


_CACHE = {}


def _get_programs(meta):
    key = (meta["NT"], meta["NB"], str(meta["chunks"]))
    if key not in _CACHE:
        progs = [_build_layer_program(meta, lay) for lay in range(3)]
        progs.append(_build_mlp_program(meta))
        _CACHE[key] = progs
    return _CACHE[key]


def run_gnn(NT=49, runner=None, **inputs):
    meta, st = _prep_inputs(inputs, NT)
    SHARD, NPAD, F = meta["SHARD"], meta["NPAD"], meta["F"]
    progs = _get_programs(meta)

    def run(nc, in_maps):
        if runner is not None:
            return runner(nc, in_maps)
        return run_bass_kernel_spmd(
            nc, in_maps, core_ids=list(range(N_CORES))).results

    def layer_maps(lay, h_in):
        maps = []
        for c in range(N_CORES):
            idx_t, dl_t, cf_t = st["core_edge"][c]
            m = {"h_in": h_in, "idx": idx_t, "dstloc": dl_t, "coef": cf_t,
                 "iota128": st["iota128"], "dinv_cols": st["dinv_cols"][c],
                 "invd_rows": st["invd_rows"][c], "W": st["W"][lay],
                 "br": st["br"][lay]}
            if lay == 0:
                m.update(ident=st["ident"], d2_rows=st["d2_rows"][c],
                         gamma=st["gamma"], beta=st["beta"])
            if lay == 2:
                m.update(iota_g=st["iota_g"], bat_cols=st["bat_cols"][c],
                         ivc_cols=st["ivc_cols"][c])
            maps.append(m)
        return maps

    h = st["x_tl"]
    for lay in range(2):
        res = run(progs[lay], layer_maps(lay, h))
        shards = [np.asarray(res[c]["h_out"]) for c in range(N_CORES)]
        h = st["tile_major"](np.concatenate(shards, axis=0))
    res = run(progs[2], layer_maps(2, h))
    pool_parts = np.concatenate(
        [np.asarray(res[c]["pool_part"]) for c in range(N_CORES)], axis=0)
    mlp_map = {"pool_parts": pool_parts, "Wc1": st["Wc1"], "Wc2": st["Wc2"],
               "bc1": st["bc1"], "bc2b": st["bc2b"]}
    res = run(progs[3], [dict(mlp_map) for _ in range(N_CORES)])
    return np.asarray(res[0]["out"], np.float32)


def kernel(**inputs):
    return run_gnn(NT=49, **inputs)


# revision 13
# speedup vs baseline: 1.1415x; 1.0611x over previous
"""Trainium2 Bass kernel for AudioOnlyGNN (3-layer GCN + BatchNorm + mean-pool + MLP).

Sharding: nodes padded to NPAD = 8*NT*128, split contiguously across the 8
NeuronCores; each core owns edges whose dst lands in its shard.  Per layer:
  - h rows gathered from HBM with dma_gather (bf16, 128-edge blocks, indices
    precomputed/permuted on host to a tile-major row order),
  - one-hot "S panels" (dst one-hot * dinv[src]) built on VectorE with one
    tensor_scalar(is_equal, mult) vs a constant iota tile,
  - TensorE matmul accumulates [F, dst-tile] PSUM tiles (contraction = edges),
  - [dst,H] = aggT @ W on TensorE + rank-1 bias matmuls,
  - ReLU with per-partition dinv[dst] scale on ScalarE.
BatchNorm is folded into W1 plus a rank-1 shift term from on-device batch
statistics (sum / sum-of-squares via matmuls).  Mean-pool uses the same
one-hot trick per dst tile.  The 3 GCN layers run as 3 SPMD launches; between
launches the host only concatenates/permutes the 8 output shards (pure data
movement) to form the replicated full-h input of the next launch.  A tiny 4th
launch sums the per-core pooled partials and runs the classifier MLP.
"""

import sys

sys.path.insert(0, "/opt/trn_rl_repo")

import contextlib

import numpy as np
import ml_dtypes

import concourse.bacc as bacc
import concourse.bass as bass
import concourse.mybir as mybir
from concourse.tile import TileContext
from concourse.bass_utils import run_bass_kernel_spmd

BF16 = mybir.dt.bfloat16
F32 = mybir.dt.float32
I16 = mybir.dt.int16

N_CORES = 8
BN_EPS = 1e-5
TILES_PER_CHUNK = 2


# ------------------------------------------------------------------ host prep
def _wrap_idx(idx):
    n = idx.shape[0]
    assert n % 16 == 0
    a = idx.astype(np.int16).reshape(n // 16, 16).T  # [16, n/16]
    return np.tile(a, (8, 1)).copy()


def _common_schedule(src, dst, NT, SHARD, JT):
    """Per-core edge lists; per-(tile, half) block counts maxed over cores so
    the SPMD program is identical on every core."""
    per_core = []
    for c in range(N_CORES):
        base = c * SHARD
        sel = (dst >= base) & (dst < base + SHARD)
        s, d = src[sel], dst[sel]
        tile = (d - base) // 128
        dloc = (d - base) % 128
        half = ((s % 128) >= 64).astype(np.int64)
        vperm = (s % 128).astype(np.int64) * JT + (s // 128)
        vrel = np.where(half == 1, vperm - 64 * JT, vperm)
        per_core.append((tile, half, vrel, dloc, s))

    nb = np.zeros((NT, 2), np.int64)
    for tile, half, _, _, _ in per_core:
        for h in (0, 1):
            cnt = np.bincount(tile[half == h], minlength=NT)
            nb[:, h] = np.maximum(nb[:, h], (cnt + 127) // 128)
    for t in range(NT):
        if nb[t].sum() == 0:
            nb[t, 0] = 1
    return per_core, nb


def _prep_core(core_data, dinv, nb, NT):
    tile, half, vrel, dloc, s = core_data
    idx_cols, dl_cols, cf_cols = [], [], []
    chunks = []
    g = 0
    for c0 in range(0, NT, TILES_PER_CHUNK):
        tsel = list(range(c0, min(c0 + TILES_PER_CHUNK, NT)))
        ch = {"g0": g, "nb": [0, 0], "tiles": {t: [] for t in tsel}}
        for h in (0, 1):
            for t in tsel:
                m = (tile == t) & (half == h)
                vr, dl = vrel[m], dloc[m]
                cf = dinv[s[m]].astype(np.float32)
                want = nb[t, h] * 128
                pad = want - vr.shape[0]
                vr = np.concatenate([vr, np.zeros(pad, np.int64)])
                dl = np.concatenate([dl, np.zeros(pad, np.int64)])
                cf = np.concatenate([cf, np.zeros(pad, np.float32)])
                for b in range(nb[t, h]):
                    ch["tiles"][t].append(g)
                    idx_cols.append(vr[b * 128:(b + 1) * 128])
                    dl_cols.append(dl[b * 128:(b + 1) * 128])
                    cf_cols.append(cf[b * 128:(b + 1) * 128])
                    ch["nb"][h] += 1
                    g += 1
        chunks.append(ch)
    idx_all = np.concatenate(idx_cols).astype(np.int16)
    dl_all = np.stack(dl_cols, axis=1).astype(np.float32)
    cf_all = np.stack(cf_cols, axis=1).astype(np.float32)
    return _wrap_idx(idx_all), dl_all, cf_all, chunks, g


# ------------------------------------------------------------------ programs
def _build_stats_program(meta):
    """Per-core BN partial sums: each core reads only its own x shard
    (tile-major compact) and emits [128, 2] = (sum x, sum x^2) per feature."""
    NT, F = meta["NT"], meta["F"]
    nc = bacc.Bacc("TRN2", target_bir_lowering=False, debug=False,
                   num_devices=N_CORES)
    xs_d = nc.dram_tensor("x_sh", [128, NT * F], BF16,
                          kind="ExternalInput").ap()
    ident_d = nc.dram_tensor("ident", [128, 128], F32,
                             kind="ExternalInput").ap()
    out_d = nc.dram_tensor("stat_part", [128, 2], F32,
                           kind="ExternalOutput").ap()
    with TileContext(nc) as tc:
        with tc.tile_pool(name="w", bufs=1) as wp, \
             tc.tile_pool(name="ps", bufs=1, space="PSUM") as pp:
            xs = wp.tile([128, NT * F], BF16, tag="xs")
            nc.sync.dma_start(out=xs[:], in_=xs_d[:])
            ident_s = wp.tile([128, 128], F32, tag="id")
            nc.sync.dma_start(out=ident_s[:], in_=ident_d[:])
            ones_s = wp.tile([128, 1], BF16, tag="ones")
            nc.vector.memset(ones_s[:], 1.0)
            xtx_ps = pp.tile([128, 128], F32, tag="xtx")
            sx_ps = pp.tile([128, 1], F32, tag="sx")
            for t in range(NT):
                sl = xs[:, t * F:(t + 1) * F]
                nc.tensor.matmul(xtx_ps[:], sl, sl, start=(t == 0),
                                 stop=(t == NT - 1))
                nc.tensor.matmul(sx_ps[:], sl, ones_s[:], start=(t == 0),
                                 stop=(t == NT - 1))
            dg = wp.tile([128, 128], F32, tag="dg")
            nc.vector.tensor_tensor(dg[:], xtx_ps[:], ident_s[:],
                                    mybir.AluOpType.mult)
            o = wp.tile([128, 2], F32, tag="o")
            nc.vector.tensor_reduce(o[:, 1:2], dg[:], mybir.AxisListType.X,
                                    mybir.AluOpType.add)
            nc.vector.tensor_copy(o[:, 0:1], sx_ps[:])
            nc.sync.dma_start(out=out_d[:], in_=o[:])
    nc.compile()
    return nc


def _build_layer_program(meta, lay):
    """One GCN layer as an SPMD program.  lay 0: BN-stats + L1; lay 1: L2;
    lay 2: L3 + pooled partials."""
    NPAD, SHARD, NT, JT, G, F, H, H2, H4, C, NB, N_true = (
        meta["NPAD"], meta["SHARD"], meta["NT"], meta["JT"], meta["G"],
        meta["F"], meta["H"], meta["H2"], meta["H4"], meta["C"],
        meta["NB"], meta["N_true"])
    chunks = meta["chunks"]
    STAT_CHUNK = 49
    HALF_ROWS = (NPAD // 128) * 64
    Ho = H if lay < 2 else H2

    nc = bacc.Bacc("TRN2", target_bir_lowering=False, debug=False,
                   num_devices=N_CORES)

    def din(name, shape, dt):
        return nc.dram_tensor(name, list(shape), dt, kind="ExternalInput").ap()

    x_d = din("h_in", [NPAD, F], BF16)
    idx_d = din("idx", [128, NB * 8], I16)
    dl_d = din("dstloc", [128, NB], F32)
    cf_d = din("coef", [128, NB], F32)
    iota_d = din("iota128", [128, 128], BF16)
    dinv_d = din("dinv_cols", [128, NT], F32)
    invd_d = din("invd_rows", [1, SHARD], BF16)
    w_d = din("W", [F, Ho], F32 if lay == 0 else BF16)
    b_d = din("br", [1, Ho], BF16)
    if lay == 0:
        sxp_d = din("sx_parts", [128, N_CORES], F32)
        exp_d = din("ex2_parts", [128, N_CORES], F32)
        d2_d = din("d2_rows", [1, SHARD], BF16)
        gam_d = din("gamma", [128, 1], F32)
        bet_d = din("beta", [128, 1], F32)
    if lay == 2:
        iotag_d = din("iota_g", [128, G], BF16)
        bat_d = din("bat_cols", [128, NT], F32)
        ivc_d = din("ivc_cols", [128, NT], F32)
        pool_out = nc.dram_tensor("pool_part", [H2, G], F32,
                                  kind="ExternalOutput").ap()
    else:
        h_out = nc.dram_tensor("h_out", [SHARD, F], BF16,
                               kind="ExternalOutput").ap()

    with TileContext(nc) as tc:
        with contextlib.ExitStack() as ctx:
            cpool = ctx.enter_context(tc.tile_pool(name="const", bufs=1))

            def cload(name, shape, dt, src):
                t = cpool.tile(list(shape), dt, tag=name)
                nc.sync.dma_start(out=t[:], in_=src)
                return t

            iota_s = cload("c_iota", [128, 128], BF16, iota_d[:])
            idx_s = cload("c_idx", [128, NB * 8], I16, idx_d[:])
            dl_s = cload("c_dl", [128, NB], F32, dl_d[:])
            cf_s = cload("c_cf", [128, NB], F32, cf_d[:])
            dinv_s = cload("c_dinv", [128, NT], F32, dinv_d[:])
            invd_s = cload("c_invd", [1, SHARD], BF16, invd_d[:])
            b_s = cload("c_b", [1, Ho], BF16, b_d[:])
            if lay == 0:
                w1f_s = cload("c_w1f", [F, H], F32, w_d[:])
                sxp_s = cload("c_sxp", [128, N_CORES], F32, sxp_d[:])
                exp_s = cload("c_exp", [128, N_CORES], F32, exp_d[:])
                d2_s = cload("c_d2", [1, SHARD], BF16, d2_d[:])
                gam_s = cload("c_gam", [128, 1], F32, gam_d[:])
                bet_s = cload("c_bet", [128, 1], F32, bet_d[:])
                w_s = cpool.tile([F, H], BF16, tag="c_wt")
                rw_s = cpool.tile([1, H], BF16, tag="c_rw")
            else:
                w_s = cload("c_w", [F, Ho], BF16, w_d[:])
            if lay == 2:
                iotag_s = cload("c_iotag", [128, G], BF16, iotag_d[:])
                bat_s = cload("c_bat", [128, NT], F32, bat_d[:])
                ivc_s = cload("c_ivc", [128, NT], F32, ivc_d[:])

            x_t = x_d.rearrange("(p j) f -> p (j f)", p=128)

            # ---- BN statistics (layer 0 only) -> W~1 and shift row rw
            if lay == 0:
                with tc.tile_pool(name="ps_st", bufs=1, space="PSUM") as pst, \
                     tc.tile_pool(name="st_w", bufs=2) as stw:
                    ex2 = stw.tile([128, 1], F32, tag="v1")
                    nc.vector.tensor_reduce(ex2[:], exp_s[:],
                                            mybir.AxisListType.X,
                                            mybir.AluOpType.add)
                    sx = stw.tile([128, 1], F32, tag="v0")
                    nc.vector.tensor_reduce(sx[:], sxp_s[:],
                                            mybir.AxisListType.X,
                                            mybir.AluOpType.add)
                    mu = stw.tile([128, 1], F32, tag="v2")
                    nc.vector.tensor_scalar_mul(mu[:], sx[:], 1.0 / N_true)
                    var = stw.tile([128, 1], F32, tag="v3")
                    nc.vector.tensor_scalar_mul(var[:], ex2[:], 1.0 / N_true)
                    mu2 = stw.tile([128, 1], F32, tag="v4")
                    nc.vector.tensor_tensor(mu2[:], mu[:], mu[:],
                                            mybir.AluOpType.mult)
                    nc.vector.tensor_tensor(var[:], var[:], mu2[:],
                                            mybir.AluOpType.subtract)
                    nc.vector.tensor_scalar_add(var[:], var[:], BN_EPS)
                    rec = stw.tile([128, 1], F32, tag="v5")
                    nc.vector.reciprocal(rec[:], var[:])
                    isd = stw.tile([128, 1], F32, tag="v6")
                    nc.scalar.activation(isd[:], rec[:],
                                         mybir.ActivationFunctionType.Sqrt)
                    a_c = stw.tile([128, 1], F32, tag="v7")
                    nc.vector.tensor_tensor(a_c[:], gam_s[:], isd[:],
                                            mybir.AluOpType.mult)
                    nc.vector.tensor_scalar_mul(w_s[:], w1f_s[:], a_c[:])
                    ca = stw.tile([128, 1], F32, tag="v8")
                    nc.vector.tensor_tensor(ca[:], mu[:], a_c[:],
                                            mybir.AluOpType.mult)
                    nc.vector.tensor_tensor(ca[:], bet_s[:], ca[:],
                                            mybir.AluOpType.subtract)
                    rw_ps = pst.tile([1, H], F32, tag="rw")
                    nc.tensor.matmul(rw_ps[:], ca[:], w1f_s[:],
                                     start=True, stop=True)
                    nc.scalar.activation(rw_s[:], rw_ps[:],
                                         mybir.ActivationFunctionType.Copy)

            # ---- the layer itself
            lay_pool = ctx.enter_context(tc.tile_pool(name="lay", bufs=3))
            sp_pool = ctx.enter_context(tc.tile_pool(name="sp", bufs=4))
            ps_agg = ctx.enter_context(
                tc.tile_pool(name="ps_agg", bufs=2, space="PSUM"))
            ps_out = ctx.enter_context(
                tc.tile_pool(name="ps_out", bufs=2, space="PSUM"))
            if lay == 2:
                ps_pl = ctx.enter_context(
                    tc.tile_pool(name="ps_pl", bufs=1, space="PSUM"))
                pool_ps = ps_pl.tile([H2, G], F32, tag="pool")

            for ch in chunks:
                nb0, nb1 = ch["nb"]
                cb = nb0 + nb1
                g0 = ch["g0"]
                hg = lay_pool.tile([128, max(cb, 1), F], BF16, tag="hg")
                if nb0:
                    nc.gpsimd.dma_gather(
                        hg[:, :nb0, :], x_d[0:HALF_ROWS, :],
                        idx_s[:, g0 * 8:(g0 + nb0) * 8],
                        nb0 * 128, nb0 * 128, F, single_packet=False)
                if nb1:
                    nc.gpsimd.dma_gather(
                        hg[:, nb0:cb, :], x_d[HALF_ROWS:NPAD, :],
                        idx_s[:, (g0 + nb0) * 8:(g0 + cb) * 8],
                        nb1 * 128, nb1 * 128, F, single_packet=False)
                for t, blocks in ch["tiles"].items():
                    agg_ps = ps_agg.tile([128, 128], F32, tag="agg")
                    for bi, g in enumerate(blocks):
                        sp = sp_pool.tile([128, 128], BF16, tag="sp")
                        nc.vector.tensor_scalar(
                            sp[:], iota_s[:], dl_s[:, g:g + 1],
                            cf_s[:, g:g + 1],
                            mybir.AluOpType.is_equal, mybir.AluOpType.mult)
                        nc.tensor.matmul(
                            agg_ps[:], hg[:, g - g0, :], sp[:],
                            start=(bi == 0), stop=(bi == len(blocks) - 1))
                    aggT = sp_pool.tile([128, 128], BF16, tag="aggT")
                    nc.scalar.activation(aggT[:], agg_ps[:],
                                         mybir.ActivationFunctionType.Copy)
                    h_ps = ps_out.tile([128, Ho], F32, tag="hps")
                    nc.tensor.matmul(h_ps[:], aggT[:], w_s[:],
                                     start=True, stop=False)
                    if lay == 0:
                        nc.tensor.matmul(
                            h_ps[:], d2_s[0:1, t * 128:(t + 1) * 128],
                            rw_s[:], start=False, stop=False)
                    nc.tensor.matmul(
                        h_ps[:], invd_s[0:1, t * 128:(t + 1) * 128],
                        b_s[:], start=False, stop=True)
                    hs = sp_pool.tile([128, Ho], BF16, tag="hs")
                    nc.scalar.activation(hs[:], h_ps[:],
                                         mybir.ActivationFunctionType.Relu,
                                         scale=dinv_s[:, t:t + 1])
                    if lay < 2:
                        nc.sync.dma_start(
                            out=h_out[t * 128:(t + 1) * 128, :], in_=hs[:])
                    else:
                        g1 = sp_pool.tile([128, G], BF16, tag="g1")
                        nc.vector.tensor_scalar(
                            g1[:], iotag_s[:], bat_s[:, t:t + 1],
                            ivc_s[:, t:t + 1],
                            mybir.AluOpType.is_equal, mybir.AluOpType.mult)
                        nc.tensor.matmul(pool_ps[:], hs[:], g1[:],
                                         start=(t == 0), stop=(t == NT - 1),
                                         skip_group_check=True)
            if lay == 2:
                po = sp_pool.tile([H2, G], F32, tag="po")
                nc.vector.tensor_copy(po[:], pool_ps[:])
                nc.sync.dma_start(out=pool_out[:], in_=po[:])

    nc.compile()
    return nc


def _build_mlp_program(meta):
    G, H2, H4, C = meta["G"], meta["H2"], meta["H4"], meta["C"]
    nc = bacc.Bacc("TRN2", target_bir_lowering=False, debug=False,
                   num_devices=N_CORES)
    pp_d = nc.dram_tensor("pool_parts", [N_CORES * H2, G], F32,
                          kind="ExternalInput").ap()
    wc1_d = nc.dram_tensor("Wc1", [H2, H4], F32, kind="ExternalInput").ap()
    wc2_d = nc.dram_tensor("Wc2", [H4, C], F32, kind="ExternalInput").ap()
    bc1_d = nc.dram_tensor("bc1", [H4, 1], F32, kind="ExternalInput").ap()
    bc2_d = nc.dram_tensor("bc2b", [G, C], F32, kind="ExternalInput").ap()
    out_d = nc.dram_tensor("out", [G, C], F32, kind="ExternalOutput").ap()

    with TileContext(nc) as tc:
        with tc.tile_pool(name="w", bufs=1) as wp, \
             tc.tile_pool(name="ps", bufs=1, space="PSUM") as pp:
            wc1_s = wp.tile([H2, H4], F32)
            nc.sync.dma_start(out=wc1_s[:], in_=wc1_d[:])
            wc2_s = wp.tile([H4, C], F32)
            nc.sync.dma_start(out=wc2_s[:], in_=wc2_d[:])
            bc1_s = wp.tile([H4, 1], F32)
            nc.sync.dma_start(out=bc1_s[:], in_=bc1_d[:])
            bc2_s = wp.tile([G, C], F32)
            nc.sync.dma_start(out=bc2_s[:], in_=bc2_d[:])
            acc = wp.tile([H2, G], F32)
            t0 = wp.tile([H2, G], F32, tag="t0")
            nc.sync.dma_start(out=acc[:], in_=pp_d[0:H2, :])
            for s in range(1, N_CORES):
                ts = wp.tile([H2, G], F32, tag=f"t{s}")
                nc.sync.dma_start(out=ts[:], in_=pp_d[s * H2:(s + 1) * H2, :])
                nc.vector.tensor_tensor(acc[:], acc[:], ts[:],
                                        mybir.AluOpType.add)
            z_ps = pp.tile([H4, G], F32, tag="z")
            nc.tensor.matmul(z_ps[:], wc1_s[:], acc[:], start=True, stop=True)
            z_s = wp.tile([H4, G], F32, tag="zs")
            nc.scalar.activation(z_s[:], z_ps[:],
                                 mybir.ActivationFunctionType.Relu,
                                 bias=bc1_s[:])
            o_ps = pp.tile([G, C], F32, tag="o")
            nc.tensor.matmul(o_ps[:], z_s[:], wc2_s[:], start=True, stop=True)
            o_s = wp.tile([G, C], F32, tag="os")
            nc.vector.tensor_tensor(o_s[:], o_ps[:], bc2_s[:],
                                    mybir.AluOpType.add)
            nc.sync.dma_start(out=out_d[:], in_=o_s[:])
    nc.compile()
    return nc


# ------------------------------------------------------------------ driver
def _prep_inputs(inputs, NT):
    x = np.asarray(inputs["x"], np.float32)
    N_true, F = x.shape
    W1 = np.asarray(inputs["W1"], np.float32)
    W2 = np.asarray(inputs["W2"], np.float32)
    W3 = np.asarray(inputs["W3"], np.float32)
    Wc1 = np.asarray(inputs["Wc1"], np.float32)
    Wc2 = np.asarray(inputs["Wc2"], np.float32)
    H, H2, H4, C = W1.shape[1], W3.shape[1], Wc1.shape[1], Wc2.shape[1]
    G = 64
    SHARD = NT * 128
    NPAD = N_CORES * SHARD
    JT = NPAD // 128

    src = np.asarray(inputs["edge_index"][0], np.int64)
    dst = np.asarray(inputs["edge_index"][1], np.int64)
    batch = np.asarray(inputs["batch"], np.int64)

    deg = np.bincount(dst, minlength=N_true).astype(np.float64) + 1.0
    dinv_t = (1.0 / np.sqrt(deg)).astype(np.float32)
    dinv = np.ones(NPAD, np.float32)
    dinv[:N_true] = dinv_t
    selfv = np.arange(N_true, dtype=np.int64)
    src_a = np.concatenate([src, selfv])
    dst_a = np.concatenate([dst, selfv])
    sneig = np.bincount(dst, weights=dinv_t[src].astype(np.float64),
                        minlength=N_true)
    # outer dinv[dst] is applied by the ReLU's per-partition scale on device
    d2_t = (sneig + dinv_t).astype(np.float32)

    per_core, nb = _common_schedule(src_a, dst_a, NT, SHARD, JT)
    core_edge, NB, chunks = [], None, None
    for c in range(N_CORES):
        idx_t, dl_t, cf_t, ch, nbt = _prep_core(per_core[c], dinv, nb, NT)
        NB = nbt
        chunks = ch
        core_edge.append((idx_t, dl_t, cf_t))

    perm = (np.arange(NPAD) % 128) * JT + (np.arange(NPAD) // 128)

    def tile_major(h_nodemajor):  # [NPAD, F] node-major -> row-permuted
        out = np.empty_like(h_nodemajor)
        out[perm] = h_nodemajor
        return out

    xp = np.zeros((NPAD, F), ml_dtypes.bfloat16)
    xp[:N_true] = x.astype(ml_dtypes.bfloat16)
    x_tl = tile_major(xp)

    iota128 = np.tile(np.arange(128, dtype=ml_dtypes.bfloat16)[None, :],
                      (128, 1)).copy()
    iota_g = np.tile(np.arange(G, dtype=ml_dtypes.bfloat16)[None, :],
                     (128, 1)).copy()

    def cols(vec, fill):
        v = np.full(NPAD, fill, np.float32)
        v[:N_true] = vec
        return v.reshape(N_CORES, NT, 128).transpose(0, 2, 1).copy()

    dinv_cols = cols(dinv_t, 1.0)
    cnt = np.bincount(batch, minlength=G).astype(np.float64)
    invc = (1.0 / np.maximum(cnt, 1.0)).astype(np.float32)
    bat_cols = cols(batch.astype(np.float32), 999.0)
    ivc_cols = cols(invc[batch], 0.0)

    def rows(vec, fill=0.0, dt=ml_dtypes.bfloat16):
        v = np.full(NPAD, fill, np.float32)
        v[:N_true] = vec
        return v.reshape(N_CORES, 1, SHARD).astype(dt)

    d2_rows = rows(d2_t)
    invd_rows = rows(np.sqrt(deg).astype(np.float32))

    meta = {"NPAD": NPAD, "SHARD": SHARD, "NT": NT, "JT": JT, "G": G,
            "F": F, "H": H, "H2": H2, "H4": H4, "C": C, "NB": NB,
            "N_true": N_true, "chunks": chunks}

    bf = ml_dtypes.bfloat16
    stat = {
        "iota128": iota128, "iota_g": iota_g,
        "ident": np.eye(128, dtype=np.float32),
        "gamma": np.asarray(inputs["bn_gamma"], np.float32).reshape(F, 1),
        "beta": np.asarray(inputs["bn_beta"], np.float32).reshape(F, 1),
        "W": [W1, W2.astype(bf), W3.astype(bf)],
        "br": [np.asarray(inputs["b1"], np.float32).reshape(1, H).astype(bf),
               np.asarray(inputs["b2"], np.float32).reshape(1, H).astype(bf),
               np.asarray(inputs["b3"], np.float32).reshape(1, H2).astype(bf)],
        "Wc1": Wc1, "Wc2": Wc2,
        "bc1": np.asarray(inputs["bc1"], np.float32).reshape(H4, 1),
        "bc2b": np.tile(np.asarray(inputs["bc2"], np.float32)[None, :],
                        (G, 1)).copy(),
        "x_tl": x_tl, "tile_major": tile_major,
        "dinv_cols": dinv_cols, "bat_cols": bat_cols, "ivc_cols": ivc_cols,
        "d2_rows": d2_rows, "invd_rows": invd_rows, "core_edge": core_edge,
    }
    return meta, stat


_CACHE = {}


def _get_programs(meta):
    key = (meta["NT"], meta["NB"], str(meta["chunks"]))
    if key not in _CACHE:
        progs = [_build_stats_program(meta)]
        progs += [_build_layer_program(meta, lay) for lay in range(3)]
        progs.append(_build_mlp_program(meta))
        _CACHE[key] = progs
    return _CACHE[key]


def run_gnn(NT=49, runner=None, **inputs):
    meta, st = _prep_inputs(inputs, NT)
    SHARD, NPAD, F = meta["SHARD"], meta["NPAD"], meta["F"]
    progs = _get_programs(meta)

    def run(nc, in_maps):
        if runner is not None:
            return runner(nc, in_maps)
        return run_bass_kernel_spmd(
            nc, in_maps, core_ids=list(range(N_CORES))).results

    # stats pre-launch: each core reads only its own shard of x
    NT, F, JT = meta["NT"], meta["F"], meta["JT"]
    x3 = st["x_tl"].reshape(128, JT, F)
    stats_maps = [{"x_sh": np.ascontiguousarray(
                       x3[:, c * NT:(c + 1) * NT, :]).reshape(128, NT * F),
                   "ident": st["ident"]} for c in range(N_CORES)]
    res = run(progs[0], stats_maps)
    parts = np.stack([np.asarray(res[c]["stat_part"]) for c in range(N_CORES)],
                     axis=2)  # [128, 2, 8]
    sx_parts = np.ascontiguousarray(parts[:, 0, :], dtype=np.float32)
    ex2_parts = np.ascontiguousarray(parts[:, 1, :], dtype=np.float32)

    def layer_maps(lay, h_in):
        maps = []
        for c in range(N_CORES):
            idx_t, dl_t, cf_t = st["core_edge"][c]
            m = {"h_in": h_in, "idx": idx_t, "dstloc": dl_t, "coef": cf_t,
                 "iota128": st["iota128"], "dinv_cols": st["dinv_cols"][c],
                 "invd_rows": st["invd_rows"][c], "W": st["W"][lay],
                 "br": st["br"][lay]}
            if lay == 0:
                m.update(sx_parts=sx_parts, ex2_parts=ex2_parts,
                         d2_rows=st["d2_rows"][c],
                         gamma=st["gamma"], beta=st["beta"])
            if lay == 2:
                m.update(iota_g=st["iota_g"], bat_cols=st["bat_cols"][c],
                         ivc_cols=st["ivc_cols"][c])
            maps.append(m)
        return maps

    h = st["x_tl"]
    for lay in range(2):
        res = run(progs[1 + lay], layer_maps(lay, h))
        shards = [np.asarray(res[c]["h_out"]) for c in range(N_CORES)]
        h = st["tile_major"](np.concatenate(shards, axis=0))
    res = run(progs[3], layer_maps(2, h))
    pool_parts = np.concatenate(
        [np.asarray(res[c]["pool_part"]) for c in range(N_CORES)], axis=0)
    mlp_map = {"pool_parts": pool_parts, "Wc1": st["Wc1"], "Wc2": st["Wc2"],
               "bc1": st["bc1"], "bc2b": st["bc2b"]}
    res = run(progs[4], [dict(mlp_map) for _ in range(N_CORES)])
    return np.asarray(res[0]["out"], np.float32)


def kernel(**inputs):
    return run_gnn(NT=49, **inputs)
